# revision 1
# baseline (speedup 1.0000x reference)
"""Trainium2 Bass kernel for nn_BenesBlock (quaternary Benes MLP-mixer block).

Strategy:
  - Data parallel: 16 examples sharded 2-per-core across 8 NeuronCores.
  - Per-example on-chip layout: SBUF tile [96 partitions (feature u), 4096 free
    (Z-order position)].  The 4-position->feature regrouping of the switch unit
    and the base-4 digit-rotation permutations (qror/qrol) are all pure
    free-dim strided access patterns in this layout, so the whole 22-layer
    stack runs out of SBUF with zero shuffle traffic.
  - Matmuls in float32r (full PE rate at N=512, near-fp32 precision),
    accumulated in fp32 PSUM.  LayerNorm(axis=positions) via bn_stats/bn_aggr
    (free-dim reduction); inv_std via bit-trick + 2 Newton iterations on DVE
    (ACT Sqrt would thrash the Gelu activation-function table, ~150us);
    Gelu tanh on ScalarE with the LN affine folded into the activation's
    per-partition scale/bias; sig*h on DVE; residual combine on GPSIMD
    writing through the permutation access pattern.  PSUM: 3 double-bank
    mm1 tiles + 2 single-bank mm2 half-tiles (8 banks exactly).
  - Per layer the two examples' mm1 phases are emitted before both mm2 phases
    so the in-order PE stream always has independent work during each
    example's LN/gelu/combine tails.
  - Z-order flatten/unflatten, feature-transpose and weight packing on host.
"""
import os
import sys
import numpy as np

for _p in ("/opt/trn_rl_repo", "/root/.axon_site/_ro/trn_rl_repo"):
    if os.path.isdir(_p) and _p not in sys.path:
        sys.path.insert(0, _p)

import concourse.bass as bass
import concourse.bacc as bacc
import concourse.mybir as mybir
import concourse.tile as tile
from concourse.bass_utils import run_bass_kernel_spmd

F32 = mybir.dt.float32
F32R = mybir.dt.float32r
I32 = mybir.dt.int32
MMDT = mybir.dt.float32r   # dtype of all matmul operands
AF = mybir.ActivationFunctionType

N_CORES = 8
B, Wd, Ht, U = 16, 64, 64, 96
N = Wd * Ht                     # 4096 positions
BPC = B // N_CORES              # 2 examples per core
L = N // 4                      # 1024 groups
U4, U8 = 4 * U, 8 * U           # 384, 768
NC1 = U8 // 128                 # 6 v-chunks for matmul1 output
LN_EPS = 1e-3
RESIDUAL_W = 0.9
CAND_W = float(np.sqrt(1.0 - RESIDUAL_W**2) * 0.25)

# layer schedule: (unit index, permutation after the switch)
LAYERS = ([(0, 'ror')] * 5 + [(1, 'rol')] * 5 + [(2, 'mid')] +
          [(3, 'ror')] * 5 + [(4, 'rol')] * 5 + [(5, 'mid')])


def _z_order_flat_idx(w, h):
    n = w * h
    k = (w - 1).bit_length()
    z = np.arange(n)
    row = np.zeros(n, np.int64)
    col = np.zeros(n, np.int64)
    for b in range(k):
        q = (z >> (2 * b)) & 3
        row |= ((q >> 1) & 1) << b
        col |= (q & 1) << b
    return row * h + col


def _r(x):
    return x.rearrange


def build_bass():
    nc = bacc.Bacc("TRN2", target_bir_lowering=False, debug=False,
                   enable_asserts=False, num_devices=N_CORES)
    xs = nc.dram_tensor("xs", [BPC, 96, N], MMDT, kind="ExternalInput").ap()
    w1 = nc.dram_tensor("w1", [6, 96, 4 * U8], MMDT, kind="ExternalInput").ap()
    w2 = nc.dram_tensor("w2", [6, 128, NC1 * U4], MMDT, kind="ExternalInput").ap()
    vg = nc.dram_tensor("vg", [96, 6 * 8], F32, kind="ExternalInput").ap()   # sig | b2c
    vl = nc.dram_tensor("vl", [128, 6 * 12 + 1], F32, kind="ExternalInput").ap()  # lnb | lnb^2+eps | rsqrt magic
    ys = nc.dram_tensor("ys", [BPC, 96, N], MMDT, kind="ExternalOutput").ap()

    with tile.TileContext(nc) as tc:
        with (
            tc.tile_pool(name="seqp", bufs=2) as seqp,
            tc.tile_pool(name="wp", bufs=2) as wp,
            tc.tile_pool(name="gp", bufs=1) as gp,
            tc.tile_pool(name="cp", bufs=1) as cp,
            tc.tile_pool(name="tp", bufs=4) as tp,
            tc.tile_pool(name="sp", bufs=8) as sp,
            tc.tile_pool(name="ps1p", bufs=3, space="PSUM") as ps1p,
            tc.tile_pool(name="ps2p", bufs=2, space="PSUM") as ps2p,
        ):
            # small per-unit constant vectors, loaded once
            vgt = cp.tile([96, 6 * 8], F32)
            nc.gpsimd.dma_start(vgt, vg)
            vlt = cp.tile([128, 6 * 12 + 1], F32)
            nc.gpsimd.dma_start(vlt, vl)

            # load both examples' sequences
            seq = []
            for ex in range(BPC):
                t = seqp.tile([96, N], MMDT, tag=f"seq{ex}", name=f"seq{ex}_in")
                nc.sync.dma_start(t, xs[ex])
                seq.append(t)

            w1t = w2t = None
            cur_unit = -1
            for li, (ui, perm) in enumerate(LAYERS):
                if ui != cur_unit:
                    cur_unit = ui
                    w1t = wp.tile([96, 4 * U8], MMDT, tag="w1", name=f"w1_{ui}")
                    nc.sync.dma_start(w1t, w1[ui])
                    w2t = wp.tile([128, NC1 * U4], MMDT, tag="w2", name=f"w2_{ui}")
                    nc.sync.dma_start(w2t, w2[ui])
                # software pipeline: emit BOTH examples' mm1 phases, then both
                # mm2+combine phases, so PE in-order stream has no gelu-tail gap
                srcs, dsts, gs = [], [], []
                for ex in range(BPC):
                    src = seq[ex]
                    dst = seqp.tile([96, N], MMDT, tag=f"seq{ex}", name=f"seq{ex}_{li}")
                    seq[ex] = dst
                    srcs.append(src)
                    dsts.append(dst)
                    # ---- matmul1 + LN + gelu -> g ----
                    g = gp.tile([128, NC1 * 1024], MMDT, tag=f"g{ex}", name=f"g_{li}_{ex}")
                    gs.append(g)
                    for c in range(NC1):
                        ps = ps1p.tile([128, 1024], F32, tag="ps1", name=f"ps1_{li}_{ex}_{c}")
                        srcv = src.rearrange("u (l s) -> u s l", s=4)
                        for j in range(4):
                            lhs = w1t[:, j * U8 + c * 128: j * U8 + (c + 1) * 128]
                            for h in range(2):
                                rhs = srcv[:, j, 512 * h: 512 * h + 512]
                                nc.tensor.matmul(
                                    ps[:, 512 * h: 512 * h + 512],
                                    lhs, rhs,
                                    start=(j == 0), stop=(j == 3))
                        st6 = sp.tile([128, 12], F32, tag="st6", name=f"st6_{li}_{ex}_{c}")
                        nc.vector.bn_stats(st6[:, 0:6], ps[:, 0:512])
                        nc.vector.bn_stats(st6[:, 6:12], ps[:, 512:1024])
                        mv = sp.tile([128, 2], F32, tag="mv", name=f"mv_{li}_{ex}_{c}")
                        nc.vector.bn_aggr(mv, st6)
                        # inv_std = 1/sqrt(var + lnb^2 + eps); bias = (lnb - mean)*inv_std
                        t0 = sp.tile([128, 1], F32, tag="t0", name=f"t0_{li}_{ex}_{c}")
                        nc.vector.tensor_add(t0, mv[:, 1:2], vlt[:, ui * 12 + 6 + c: ui * 12 + 7 + c])
                        # rsqrt(t0) via bit-trick + 2 Newton iterations (all DVE,
                        # avoids ACT Sqrt which would thrash the Gelu act table)
                        sh = sp.tile([128, 1], F32, tag="sh", name=f"sh_{li}_{ex}_{c}")
                        nc.vector.tensor_scalar(sh.bitcast(I32), t0.bitcast(I32), 1, None,
                                                op0=mybir.AluOpType.arith_shift_right)
                        y0 = sp.tile([128, 1], F32, tag="y0", name=f"y0_{li}_{ex}_{c}")
                        nc.vector.tensor_tensor(y0.bitcast(I32), vlt[:, 72:73].bitcast(I32),
                                                sh.bitcast(I32), op=mybir.AluOpType.subtract)
                        kf = sp.tile([128, 1], F32, tag="kf", name=f"kf_{li}_{ex}_{c}")
                        nc.vector.tensor_scalar(kf, t0, -0.5, None, op0=mybir.AluOpType.mult)
                        yy = y0
                        for it in range(2):
                            aa = sp.tile([128, 1], F32, tag=f"aa{it}", name=f"aa{it}_{li}_{ex}_{c}")
                            nc.vector.tensor_mul(aa, yy, yy)
                            bb = sp.tile([128, 1], F32, tag=f"bb{it}", name=f"bb{it}_{li}_{ex}_{c}")
                            nc.vector.tensor_scalar(bb, aa, kf, 1.5,
                                                    op0=mybir.AluOpType.mult, op1=mybir.AluOpType.add)
                            y2 = sp.tile([128, 1], F32, tag=f"y2{it}", name=f"y2{it}_{li}_{ex}_{c}")
                            nc.vector.tensor_mul(y2, yy, bb)
                            yy = y2
                        invs = yy
                        bia = sp.tile([128, 1], F32, tag="bia", name=f"bia_{li}_{ex}_{c}")
                        nc.vector.tensor_scalar(
                            bia, vlt[:, ui * 12 + c: ui * 12 + c + 1],
                            mv[:, 0:1], invs,
                            op0=mybir.AluOpType.subtract, op1=mybir.AluOpType.mult)
                        nc.scalar.activation(
                            g[:, c * 1024: (c + 1) * 1024], ps,
                            AF.Gelu_apprx_tanh, bias=bia, scale=invs)
                for ex in range(BPC):
                    src, dst, g = srcs[ex], dsts[ex], gs[ex]
                    # ---- matmul2 + combine + permuted write ----
                    for j in range(4):
                        tmp = tp.tile([96, 1024], F32, tag="tmp", name=f"tmp_{li}_{ex}_{j}")
                        for h in range(2):
                            ps2 = ps2p.tile([96, 512], F32, tag="ps2", name=f"ps2_{li}_{ex}_{j}_{h}")
                            for c in range(NC1):
                                lhs2 = w2t[:, c * U4 + j * 96: c * U4 + (j + 1) * 96]
                                nc.tensor.matmul(
                                    ps2, lhs2,
                                    g[:, c * 1024 + 512 * h: c * 1024 + 512 * h + 512],
                                    start=(c == 0), stop=(c == NC1 - 1))
                            # tmp = ps2 + b2c_j   (ACT affine, PSUM->SBUF, per half)
                            nc.scalar.activation(tmp[:, 512 * h: 512 * h + 512], ps2, AF.Identity,
                                                 bias=vgt[:, ui * 8 + 4 + j: ui * 8 + 5 + j])
                        # u = sig_j * h_j  (DVE, strided read; keeps ACT free for
                        # the tmp/gelu chain so the combine lands earlier)
                        um = tp.tile([96, 1024], F32, tag="um", name=f"um_{li}_{ex}_{j}")
                        nc.vector.tensor_scalar(
                            um, src.rearrange("u (l s) -> u s l", s=4)[:, j, :],
                            vgt[:, ui * 8 + j: ui * 8 + j + 1], None,
                            op0=mybir.AluOpType.mult)
                        # out = tmp + u, written through the permutation AP
                        if perm == 'ror':
                            dstv = dst.rearrange("u (t s) -> u s t", s=16)[:, 4 * j: 4 * j + 4, :]
                            nc.gpsimd.tensor_add(
                                dstv,
                                tmp.rearrange("u (a t) -> u a t", a=4),
                                um.rearrange("u (a t) -> u a t", a=4))
                        elif perm == 'rol':
                            nc.gpsimd.tensor_add(dst[:, j * 1024: (j + 1) * 1024], tmp, um)
                        else:
                            dstv = dst.rearrange("u (l s) -> u s l", s=4)[:, j, :]
                            nc.gpsimd.tensor_add(dstv, tmp, um)
            for ex in range(BPC):
                nc.sync.dma_start(ys[ex], seq[ex])
    if not nc.is_finalized():
        nc.finalize()
    return nc


_CACHED = {}


def _get_nc():
    if "nc" not in _CACHED:
        _CACHED["nc"] = build_bass()
    return _CACHED["nc"]


def _pack_inputs(x, W1, ln_bias, W2, b2, res_scale):
    x = np.ascontiguousarray(np.asarray(x, np.float32))
    W1 = np.asarray(W1, np.float32)
    W2 = np.asarray(W2, np.float32)
    b2 = np.asarray(b2, np.float32)
    ln_bias = np.asarray(ln_bias, np.float32)
    res_scale = np.asarray(res_scale, np.float32)

    flat = _z_order_flat_idx(Wd, Ht)
    seqT = x.reshape(B, N, U)[:, flat].transpose(0, 2, 1)
    seqT = np.ascontiguousarray(seqT)                       # [16, 96, 4096]

    W1p = np.stack([W1[b, k].reshape(4, 96, U8).transpose(1, 0, 2).reshape(96, 4 * U8)
                    for b in range(2) for k in range(3)])
    W2p = np.stack([(W2[b, k] * CAND_W).reshape(NC1, 128, U4).transpose(1, 0, 2).reshape(128, NC1 * U4)
                    for b in range(2) for k in range(3)])
    sig = np.stack([(1.0 / (1.0 + np.exp(-res_scale[b, k]))).reshape(4, 96).T
                    for b in range(2) for k in range(3)])
    b2c = np.stack([(CAND_W * b2[b, k]).reshape(4, 96).T
                    for b in range(2) for k in range(3)])
    vgp = np.ascontiguousarray(
        np.concatenate([sig, b2c], axis=2).transpose(1, 0, 2).reshape(96, 48))
    lnbp = np.stack([ln_bias[b, k].reshape(NC1, 128).T
                     for b in range(2) for k in range(3)])
    vlp = np.concatenate([lnbp, lnbp**2 + LN_EPS], axis=2).transpose(1, 0, 2).reshape(128, 72)
    magic = np.full((128, 1), np.uint32(0x5f3759df), np.uint32).view(np.float32)
    vlp = np.ascontiguousarray(np.concatenate([vlp, magic], axis=1))
    return seqT, W1p, W2p, vgp, vlp


def kernel(x, W1, ln_bias, W2, b2, res_scale, _trace=False, _tmpdir=None):
    seqT, W1p, W2p, vgp, vlp = _pack_inputs(x, W1, ln_bias, W2, b2, res_scale)
    nc = _get_nc()
    in_maps = []
    for core in range(N_CORES):
        in_maps.append({
            "xs": np.ascontiguousarray(seqT[core * BPC:(core + 1) * BPC]),
            "w1": W1p, "w2": W2p, "vg": vgp, "vl": vlp,
        })
    res = run_bass_kernel_spmd(nc, in_maps, core_ids=list(range(N_CORES)),
                               trace=_trace, tmpdir=_tmpdir,
                               stitch_traces=False)
    outT = np.concatenate([res.results[c]["ys"] for c in range(N_CORES)], axis=0)

    flat = _z_order_flat_idx(Wd, Ht)
    inv = np.argsort(flat)
    out = outT.transpose(0, 2, 1)[:, inv].reshape(B, Wd, Ht, U)
    if _trace:
        return np.ascontiguousarray(out.astype(np.float32)), res
    return np.ascontiguousarray(out.astype(np.float32))

